# revision 21
# baseline (speedup 1.0000x reference)
"""MGU RNN (nn_Network_82394652607110) — Trainium2 Bass kernel, v4.

v3 (176935 ns) -> v4 changes, from trace analysis:
 - Host pre-transposes tx, so phase 1 loads are plain contiguous
   [128, 2048] DMAs spread across BOTH HWDGE rings instead of 16
   serialized xbar transposes (xbar transposes occupy the issuing
   engine for the full transfer: 2.07us each, one ring only -> 33us
   of Scalar-engine time + a WAR cascade that stretched phase 1 to
   80us and delayed sweep-0's sigmoid to 78us).
 - Block-major emission: each block's groups are followed by its
   sweep-0 unit, so the sweep pipeline starts as soon as block 0's
   P12 lands (~12us) and overlaps the rest of phase 1.
 - w = 1 - sigmoid(pa) computed as sigmoid(-pa) on ACT (activation
   scale=-1), moving ~2.7us/sweep off the Vector engine (the phase-2
   bottleneck at ~80% busy).
 - Head reworked: logits_g = sum_u h_T[g,u] fc_w[u,:] + fc_b computed
   as a selector matmul (lhsT = M2 [126, 25] with a ones bias row,
   rhs = fcw125 * Hb[:, T] built by one DVE op per block), replacing
   15 tiny partition-strided gather DMAs (~5us tail).
 - Memsets trimmed to P12 dead lanes (32-aligned bases) + Hb col 0.

Kept from v3 (measured hazards -- avoid regressing!):
 - DMA APs with >=2 partition dims mis-lower -> remaps stay one
   contiguous 20-row partition run; engine ops need 32-aligned
   partition bases; matmul psum base must be 0/32/64; gpsimd cannot
   read PSUM; gpsimd has no scan.
 - Quasi-DEER: NSWEEPS=6 (deterministic max err 9.5e-3, L2 6.9e-4 vs
   the 2e-2 gate). Sweep 0 specializes h=0. Matmuls batched per
   weight ACROSS blocks; psum drains for phase 1 on DVE; scans in
   2x512 chunks (a single 1024-col scan runs at 4 cyc/col vs 2.5).

Layout: per block bl in {0,1,2}: P12[bl] [125, 2048] fp16 (p1 cols
0..1024, p2 cols 1024..2048), partitions 5g+u, batch b = B0[bl]+g,
live groups 24/24/16 of 25. Hb[bl] [125, 1088] fp16: col 0 = zero
initial state, scan writes 1..1024.
"""

import os
import numpy as np

import concourse.bass as bass
import concourse.bacc as bacc
import concourse.tile as tile
import concourse.mybir as mybir
from concourse.bass_utils import run_bass_kernel_spmd

dt = mybir.dt
AF = mybir.ActivationFunctionType
ALU = mybir.AluOpType

# Problem constants (hardcoded per harness contract)
U = 5
T = 1024
D = 64
B = 512
NCORES = 8
BC = B // NCORES          # 64 batch per core
NPAIR = BC // 2           # 32
NLOAD = NPAIR // 2        # 16 loads, two pairs each

G = 25                    # partition groups per block
P = G * U                 # 125 partitions
BL = 3                    # blocks
B0 = [0, 24, 48]          # first batch of each block
NB = [24, 24, 16]         # live batches (groups) per block
# 6-batch psum groups (3 pairs at bases 0/32/64); last group has 2 pairs
GRP_BL = [0, 0, 0, 0, 1, 1, 1, 1, 2, 2, 2]
GRP_G0 = [0, 6, 12, 18, 0, 6, 12, 18, 0, 6, 12]
GRP_NP = [3, 3, 3, 3, 3, 3, 3, 3, 3, 3, 2]   # pairs per group
BL_GRPS = [[0, 1, 2, 3], [4, 5, 6, 7], [8, 9, 10]]

NSWEEPS = int(os.environ.get("MGU_NSWEEPS", "6"))
MM_DT = dt.float16
F16 = dt.float16
F32 = dt.float32


def build_program():
    nc = bacc.Bacc("TRN2", target_bir_lowering=False, debug=False)

    # pre-transposed tx: [load, (b01 d), (q_lo t)]
    txpt = nc.dram_tensor("txpt", [NLOAD, 2 * D, 2 * T], F16,
                          kind="ExternalInput")
    khp = nc.dram_tensor("khp", [2 * D, 32], F16, kind="ExternalInput")
    b128 = nc.dram_tensor("b128", [128, 1], F32, kind="ExternalInput")
    bd_rf = nc.dram_tensor("bd_rf", [P, P], MM_DT, kind="ExternalInput")
    bd_rh = nc.dram_tensor("bd_rh", [P, P], MM_DT, kind="ExternalInput")
    ident = nc.dram_tensor("ident", [P, P], MM_DT, kind="ExternalInput")
    m2 = nc.dram_tensor("m2", [P + 1, G], F16, kind="ExternalInput")
    fcw125 = nc.dram_tensor("fcw125", [P, 4], F16, kind="ExternalInput")
    fcb = nc.dram_tensor("fcb", [1, 4], F16, kind="ExternalInput")
    out = nc.dram_tensor("out", [BC, 4], F32, kind="ExternalOutput")
    dbg = os.environ.get("MGU_DEBUG_DUMP", "0") == "1"
    if dbg:
        p12d = [nc.dram_tensor(f"p12d_{b}", [P, 2 * T], F16,
                               kind="ExternalOutput") for b in range(BL)]
        hbd = [nc.dram_tensor(f"hbd_{b}", [P, T + 64], F16,
                              kind="ExternalOutput") for b in range(BL)]

    with tile.TileContext(nc) as tc:
        with (
            tc.tile_pool(name="consts", bufs=1) as consts,
            tc.tile_pool(name="master", bufs=1) as master,
            tc.tile_pool(name="xt", bufs=16) as xt_pool,
            tc.tile_pool(name="stg", bufs=6) as stg_pool,
            tc.tile_pool(name="ps1", bufs=2, space="PSUM") as ps1_pool,
            tc.tile_pool(name="ps2", bufs=3, space="PSUM") as ps2_pool,
            tc.tile_pool(name="gv1", bufs=3) as gv1_pool,
            tc.tile_pool(name="gw", bufs=3) as gw_pool,
            tc.tile_pool(name="ghv", bufs=3) as ghv_pool,
            tc.tile_pool(name="gv2", bufs=3) as gv2_pool,
            tc.tile_pool(name="gm", bufs=3) as gm_pool,
            tc.tile_pool(name="head", bufs=1) as head_pool,
        ):
            # ---- persistent master-layout tensors (allocated first so
            # the gpsimd dead-lane memsets can precede the const DMAs) ----
            P12 = [master.tile([P, 2 * T], F16, tag=f"P12_{b}", name=f"P12_{b}")
                   for b in range(BL)]
            Hb = [master.tile([P, T + 64], F16, tag=f"Hb_{b}", name=f"Hb_{b}")
                  for b in range(BL)]
            # dead lanes (g >= NB[bl]) must be ZERO: the block-diag matmuls
            # multiply every lane by the weight column (0 * NaN = NaN would
            # pollute live psum rows). 32-aligned bases; on gpsimd so the
            # DVE queue is free for the phase-1 drains.
            nc.vector.memset(P12[0][96:P, :], 0.0)
            nc.vector.memset(P12[1][96:P, :], 0.0)
            nc.vector.memset(P12[2][64:P, :], 0.0)
            for b in range(BL):
                nc.vector.memset(Hb[b][:, 0:1], 0.0)   # h0 = 0

            # ---- constants to SBUF on the gpsimd (SWDGE) ring, keeping
            # both HWDGE rings free for the tx loads + remaps ----
            khp_sb = consts.tile([2 * D, 32], F16, tag="khp")
            b128_sb = consts.tile([128, 1], F32, tag="b128")
            bdrf_sb = consts.tile([P, P], MM_DT, tag="bdrf")
            bdrh_sb = consts.tile([P, P], MM_DT, tag="bdrh")
            id_sb = consts.tile([P, P], MM_DT, tag="ident")
            m2_sb = consts.tile([P + 1, G], F16, tag="m2")
            fcw_sb = consts.tile([P, 4], F16, tag="fcw125")
            nc.gpsimd.dma_start(khp_sb[:], khp[:])
            nc.gpsimd.dma_start(b128_sb[:], b128[:])
            nc.gpsimd.dma_start(id_sb[:], ident[:])
            nc.gpsimd.dma_start(bdrf_sb[:], bd_rf[:])
            nc.gpsimd.dma_start(bdrh_sb[:], bd_rh[:])
            nc.gpsimd.dma_start(m2_sb[:], m2[:])
            nc.gpsimd.dma_start(fcw_sb[:], fcw125[:])
            # head rhs tiles: rows 0..124 written per block at the final
            # sweep; row 125 = fc_b (ones row of m2 adds the bias)
            rhs2 = [head_pool.tile([P + 1, 4], F16, tag=f"rhs2_{b}",
                                   name=f"rhs2_{b}") for b in range(BL)]
            for b in range(BL):
                nc.gpsimd.dma_start(rhs2[b][P:P + 1, :], fcb[:])
            # preload the Exp ACT table now (1.28us); otherwise it loads
            # lazily right before the head's exp, on the tail critical path
            exd = head_pool.tile([1, 1], F32, tag="exd")
            nc.scalar.activation(exd[:], b128_sb[0:1, 0:1], AF.Exp)

            # ---- Phase 1: plain transposed loads + projection ----
            # Loads are emitted per block (see the emission loop below):
            # the DMA engines are a single globally-serialized resource
            # (~650ns issue + bytes/360GBps per instruction), so block 0's
            # remaps must not queue behind later blocks' loads.
            xt2s = {}

            def emit_loads(bl):
                for qq in range(*([0, 6], [6, 12], [12, 16])[bl]):
                    xt = xt_pool.tile([2 * D, 2 * T], F16, tag="xt",
                                      name="xt")
                    eng = nc.sync if qq % 2 == 0 else nc.scalar
                    eng.dma_start(out=xt[:], in_=txpt[qq])
                    xt2s[qq] = xt

            def xt_slice(q, th):
                return xt2s[q // 2][:, (q % 2) * T + th * 512:
                                    (q % 2) * T + th * 512 + 512]

            remap_cnt = [0]

            def emit_group(grp):
                bl = GRP_BL[grp]
                g0 = GRP_G0[grp]
                np_ = GRP_NP[grp]
                q0 = 3 * grp
                stg = stg_pool.tile([128, 2 * 512], F16, tag="stg")
                for th in range(2):
                    ps = ps1_pool.tile([128, 512], F32, tag="psA")
                    for ql in range(np_):
                        nc.tensor.matmul(
                            ps[32 * ql:32 * ql + 32, :],
                            lhsT=khp_sb[:],
                            rhs=xt_slice(q0 + ql, th),
                            start=True, stop=True,
                        )
                    nrow = 32 * np_
                    # drains on DVE: keeps the scalar queue free for the
                    # sweep activations
                    nc.vector.tensor_scalar(
                        stg[:nrow, 512 * th:512 * th + 512], ps[:nrow, :],
                        b128_sb[:nrow, :], None, ALU.add)
                # remap (both th halves merged -> 33 DMAs total; each DMA
                # instruction costs ~650-784ns of serial ring issue):
                # src rows 32*ql + 2*(5*b01+u) + gate (contig 20), free
                # (th, t) -> P12[bl] partition 5*(g0 + 2*ql + b01) + u,
                # free col gate*1024 + th*512 + t.
                for ql in range(np_):
                    s_ap = stg[32 * ql:32 * ql + 20, :]
                    d_ap = (P12[bl][5 * (g0 + 2 * ql):
                                    5 * (g0 + 2 * ql) + 10, :]
                            .rearrange("p (gate tt t) -> p gate tt t",
                                       gate=2, tt=2))
                    eng = (nc.sync, nc.scalar, nc.gpsimd)[remap_cnt[0] % 3]
                    remap_cnt[0] += 1
                    eng.dma_start(out=d_ap, in_=s_ap)

            # ---- Phase 2 helpers ----
            def mm_pair(ps_t, w_sb, rhs_full, start):
                # accumulate w_sb.T @ rhs into ps_t ([P, T]); 512-col halves
                # (a single 1024-col matmul crosses a psum bank -> illegal)
                for c in range(2):
                    sl = slice(c * 512, (c + 1) * 512)
                    nc.tensor.matmul(ps_t[:, sl], lhsT=w_sb[:],
                                     rhs=rhs_full[:, sl],
                                     start=start, stop=not start)

            def scan_block(bl, w, m):
                # h[t] = w[t]*h[t-1] + m[t], fp32 state (DVE only).
                nc.vector.tensor_tensor_scan(
                    Hb[bl][:, 1:513], w[:, 0:512], m[:, 0:512],
                    0.0, ALU.mult, ALU.add)
                nc.vector.tensor_tensor_scan(
                    Hb[bl][:, 513:T + 1], w[:, 512:T], m[:, 512:T],
                    Hb[bl][:, 512:513], ALU.mult, ALU.add)

            def emit_head(bl):
                # logits = M2.T @ (fcw125 * h_T ++ fc_b); per-block head +
                # softmax + out DMA so block 0's output leaves while block
                # 2 is still scanning (the shared tail measured ~7us).
                # (tensor_scalar wants an f32 scalar AP -> cast h_T col)
                hcol = head_pool.tile([P, 1], F32, tag=f"hcol_{bl}",
                                      name=f"hcol_{bl}")
                nc.vector.tensor_scalar(hcol[:], Hb[bl][:, T:T + 1],
                                        1.0, None, ALU.mult)
                nc.vector.tensor_scalar(rhs2[bl][0:P, :], fcw_sb[:],
                                        hcol[:], None, ALU.mult)
                pl = ps1_pool.tile([G, 4], F32, tag="psA", name=f"pl_{bl}")
                nc.tensor.matmul(pl[:], lhsT=m2_sb[:], rhs=rhs2[bl][:],
                                 start=True, stop=True)
                # |logits| < ~3: exp cannot overflow f32 -> skip max-shift
                ex = head_pool.tile([G, 4], F32, tag=f"ex_{bl}",
                                    name=f"ex_{bl}")
                sm = head_pool.tile([G, 1], F32, tag=f"sm_{bl}",
                                    name=f"sm_{bl}")
                nc.scalar.activation(ex[:], pl[:], AF.Exp, accum_out=sm[:])
                ri = head_pool.tile([G, 1], F32, tag=f"ri_{bl}",
                                    name=f"ri_{bl}")
                nc.vector.reciprocal(ri[:], sm[:])
                op = head_pool.tile([G, 4], F32, tag=f"op_{bl}",
                                    name=f"op_{bl}")
                nc.vector.tensor_scalar(op[:], ex[:], ri[:], None, ALU.mult)
                eng = (nc.sync, nc.scalar, nc.gpsimd)[bl]
                eng.dma_start(out=out[B0[bl]:B0[bl] + NB[bl], :],
                              in_=op[0:NB[bl], :])

            def emit_sweep0(bl):
                # sweep 0: h == 0 -> pa = P1, pb = P2, no matmuls
                v1 = gv1_pool.tile([P, T], F16, tag="v1", name="v1")
                nc.scalar.activation(v1[:], P12[bl][:, 0:T], AF.Sigmoid)
                v2 = gv2_pool.tile([P, T], F16, tag="v2", name="v2")
                nc.scalar.activation(v2[:], P12[bl][:, T:2 * T], AF.Tanh)
                # w = 1 - v1 on gpsimd (idle engine; ACT is the phase-2
                # bottleneck and w is off the critical chain)
                w = gw_pool.tile([P, T], F16, tag="w", name="w")
                nc.gpsimd.tensor_scalar(w[:], v1[:], -1.0, 1.0,
                                        ALU.mult, ALU.add)
                m = gm_pool.tile([P, T], F16, tag="m", name="m")
                nc.vector.tensor_tensor(m[:], v1[:], v2[:], ALU.mult)
                scan_block(bl, w, m)

            # ---- emission ----
            # Block-major phase 1 + sweep 0: block bl's sweep-0 unit starts
            # as soon as its last remap lands, overlapping later blocks'
            # loads/projections.
            for bl in range(BL):
                emit_loads(bl)
                for grp in BL_GRPS[bl]:
                    emit_group(grp)
                emit_sweep0(bl)

            # Lockstep sweeps with cross-block weight batching (PE
            # pipelining; per-unit emission measured 60us slower on v2).
            for s in range(1, NSWEEPS):
                # per-block MM interleave: pa[bl] completes after ITS 4
                # MMs instead of waiting the whole cross-block batch (the
                # batched order made pb[0] transitively wait on hv[2],
                # stretching the sweep cadence to ~15.5us vs ~11us busy)
                pa = [ps2_pool.tile([P, T], F32, tag="ps2", name="pa")
                      for _ in range(BL)]
                for bl in range(BL):
                    mm_pair(pa[bl], bdrf_sb, Hb[bl][:, 0:T], start=True)
                    mm_pair(pa[bl], id_sb, P12[bl][:, 0:T], start=False)
                v1s, ws, hvs = [], [], []
                for bl in range(BL):
                    v1 = gv1_pool.tile([P, T], F16, tag="v1", name="v1")
                    nc.scalar.activation(v1[:], pa[bl][:], AF.Sigmoid)
                    v1s.append(v1)
                    hv = ghv_pool.tile([P, T], F16, tag="hv", name="hv")
                    nc.vector.tensor_tensor(hv[:], Hb[bl][:, 0:T], v1[:],
                                            ALU.mult)
                    hvs.append(hv)
                # w = 1 - v1 on gpsimd: off the critical chain (only the
                # scan reads it) and keeps the saturated ACT queue free
                # for the tanhs
                for bl in range(BL):
                    w = gw_pool.tile([P, T], F16, tag="w", name="w")
                    nc.gpsimd.tensor_scalar(w[:], v1s[bl][:], -1.0, 1.0,
                                            ALU.mult, ALU.add)
                    ws.append(w)
                pb = [ps2_pool.tile([P, T], F32, tag="ps2", name="pb")
                      for _ in range(BL)]
                for bl in range(BL):
                    mm_pair(pb[bl], bdrh_sb, hvs[bl][:], start=True)
                    mm_pair(pb[bl], id_sb, P12[bl][:, T:2 * T], start=False)
                for bl in range(BL):
                    v2 = gv2_pool.tile([P, T], F16, tag="v2", name="v2")
                    nc.scalar.activation(v2[:], pb[bl][:], AF.Tanh)
                    m = gm_pool.tile([P, T], F16, tag="m", name="m")
                    nc.vector.tensor_tensor(m[:], v1s[bl][:], v2[:],
                                            ALU.mult)
                    scan_block(bl, ws[bl], m)
                    if s == NSWEEPS - 1:
                        emit_head(bl)

            if dbg:
                for b in range(BL):
                    nc.gpsimd.dma_start(out=p12d[b][:], in_=P12[b][:])
                    nc.gpsimd.dma_start(out=hbd[b][:], in_=Hb[b][:])

    nc.compile()
    return nc


def _prep_host_inputs(kernel, rec_kernel, bias, fc_w, fc_b):
    f32 = np.float32
    k = np.asarray(kernel, f32).astype(np.float16)    # [64, 10]

    # psum row (within a 32-row pair slot) = 2*(5*b01 + u) + gate
    # (gate innermost so the remap DMA sees one contiguous 20-row run)
    khp = np.zeros((2 * D, 32), np.float16)
    b128 = np.zeros((128, 1), f32)
    bias_f = np.asarray(bias, f32)
    for gate in range(2):
        for b01 in range(2):
            for u in range(U):
                c = 2 * (5 * b01 + u) + gate
                khp[D * b01:D * b01 + D, c] = k[:, 5 * gate + u]
                for ql in range(4):
                    b128[32 * ql + c, 0] = bias_f[5 * gate + u]

    rk = np.asarray(rec_kernel, f32)
    bd_rf = np.zeros((P, P), np.float16)
    bd_rh = np.zeros((P, P), np.float16)
    for g in range(G):
        bd_rf[5 * g:5 * g + 5, 5 * g:5 * g + 5] = rk[:, :U]
        bd_rh[5 * g:5 * g + 5, 5 * g:5 * g + 5] = rk[:, U:]
    ident = np.eye(P, dtype=np.float16)

    # head selector: logits[g, j] = sum_u h[5g+u] fc_w[u, j] + fc_b[j]
    m2 = np.zeros((P + 1, G), np.float16)
    for g in range(G):
        m2[5 * g:5 * g + 5, g] = 1.0
    m2[P, :] = 1.0
    fcw125 = np.tile(np.asarray(fc_w, f32), (G, 1)).astype(np.float16)
    fcb = np.asarray(fc_b, f32).reshape(1, 4).astype(np.float16)
    return dict(khp=khp, b128=b128, bd_rf=bd_rf, bd_rh=bd_rh, ident=ident,
                m2=m2, fcw125=fcw125, fcb=fcb)


_CACHE = {}


def kernel(tx, kernel, rec_kernel, bias, fc_w, fc_b, _want_time=False):
    tx = np.asarray(tx, np.float32)
    host = _prep_host_inputs(kernel, rec_kernel, bias, fc_w, fc_b)

    # fp16 pre-transposed tx: [core, load, (b01, d), (q_lo, t)]
    # load qq covers pairs 2qq, 2qq+1; pair pq covers batches 2pq, 2pq+1.
    txpt_all = np.ascontiguousarray(
        tx.reshape(NCORES, NLOAD, 2, 2, T, D)    # c, qq, q_lo, b01, t, d
        .transpose(0, 1, 3, 5, 2, 4)             # c, qq, b01, d, q_lo, t
        .reshape(NCORES, NLOAD, 2 * D, 2 * T).astype(np.float16))

    if "nc" not in _CACHE:
        _CACHE["nc"] = build_program()
    nc = _CACHE["nc"]

    in_maps = []
    for c in range(NCORES):
        m = {"txpt": txpt_all[c]}
        m.update(host)
        in_maps.append(m)

    try:
        res = run_bass_kernel_spmd(
            nc, in_maps, core_ids=list(range(NCORES)), trace=_want_time
        )
    except ModuleNotFoundError:
        res = run_bass_kernel_spmd(
            nc, in_maps, core_ids=list(range(NCORES)), trace=False
        )
    outs = [res.results[c]["out"] for c in range(NCORES)]
    full = np.concatenate(outs, axis=0)
    if _want_time:
        _CACHE["res"] = res
        return full, res.exec_time_ns
    return full


# revision 29
# speedup vs baseline: 1.1102x; 1.1102x over previous
"""MGU RNN (nn_Network_82394652607110) — Trainium2 Bass kernel, v4.

v3 (176935 ns) -> v4 changes, from trace analysis:
 - Host pre-transposes tx, so phase 1 loads are plain contiguous
   [128, 2048] DMAs spread across BOTH HWDGE rings instead of 16
   serialized xbar transposes (xbar transposes occupy the issuing
   engine for the full transfer: 2.07us each, one ring only -> 33us
   of Scalar-engine time + a WAR cascade that stretched phase 1 to
   80us and delayed sweep-0's sigmoid to 78us).
 - Block-major emission: each block's groups are followed by its
   sweep-0 unit, so the sweep pipeline starts as soon as block 0's
   P12 lands (~12us) and overlaps the rest of phase 1.
 - w = 1 - sigmoid(pa) computed as sigmoid(-pa) on ACT (activation
   scale=-1), moving ~2.7us/sweep off the Vector engine (the phase-2
   bottleneck at ~80% busy).
 - Head reworked: logits_g = sum_u h_T[g,u] fc_w[u,:] + fc_b computed
   as a selector matmul (lhsT = M2 [126, 25] with a ones bias row,
   rhs = fcw125 * Hb[:, T] built by one DVE op per block), replacing
   15 tiny partition-strided gather DMAs (~5us tail).
 - Memsets trimmed to P12 dead lanes (32-aligned bases) + Hb col 0.

Kept from v3 (measured hazards -- avoid regressing!):
 - DMA APs with >=2 partition dims mis-lower -> remaps stay one
   contiguous 20-row partition run; engine ops need 32-aligned
   partition bases; matmul psum base must be 0/32/64; gpsimd cannot
   read PSUM; gpsimd has no scan.
 - Quasi-DEER: NSWEEPS=6 (deterministic max err 9.5e-3, L2 6.9e-4 vs
   the 2e-2 gate). Sweep 0 specializes h=0. Matmuls batched per
   weight ACROSS blocks; psum drains for phase 1 on DVE; scans in
   2x512 chunks (a single 1024-col scan runs at 4 cyc/col vs 2.5).

Layout: per block bl in {0,1,2}: P12[bl] [125, 2048] fp16 (p1 cols
0..1024, p2 cols 1024..2048), partitions 5g+u, batch b = B0[bl]+g,
live groups 24/24/16 of 25. Hb[bl] [125, 1088] fp16: col 0 = zero
initial state, scan writes 1..1024.
"""

import os
import numpy as np

import concourse.bass as bass
import concourse.bacc as bacc
import concourse.tile as tile
import concourse.mybir as mybir
from concourse.bass_utils import run_bass_kernel_spmd

dt = mybir.dt
AF = mybir.ActivationFunctionType
ALU = mybir.AluOpType

# Problem constants (hardcoded per harness contract)
U = 5
T = 1024
D = 64
B = 512
NCORES = 8
BC = B // NCORES          # 64 batch per core
NPAIR = BC // 2           # 32
NLOAD = NPAIR // 2        # 16 loads, two pairs each

G = 25                    # partition groups per block
P = G * U                 # 125 partitions
BL = 3                    # blocks
B0 = [0, 24, 48]          # first batch of each block
NB = [24, 24, 16]         # live batches (groups) per block
# 6-batch psum groups (3 pairs at bases 0/32/64); last group has 2 pairs
GRP_BL = [0, 0, 0, 0, 1, 1, 1, 1, 2, 2, 2]
GRP_G0 = [0, 6, 12, 18, 0, 6, 12, 18, 0, 6, 12]
GRP_NP = [3, 3, 3, 3, 3, 3, 3, 3, 3, 3, 2]   # pairs per group
BL_GRPS = [[0, 1, 2, 3], [4, 5, 6, 7], [8, 9, 10]]

NSWEEPS = int(os.environ.get("MGU_NSWEEPS", "6"))
MM_DT = dt.float16
F16 = dt.float16
F32 = dt.float32


def build_program():
    nc = bacc.Bacc("TRN2", target_bir_lowering=False, debug=False)

    # pre-transposed tx: [load, (b01 d), (q_lo t)]
    txpt = nc.dram_tensor("txpt", [NLOAD, 2 * D, 2 * T], F16,
                          kind="ExternalInput")
    khp = nc.dram_tensor("khp", [2 * D, 32], F16, kind="ExternalInput")
    b128 = nc.dram_tensor("b128", [128, 1], F32, kind="ExternalInput")
    bd_rf = nc.dram_tensor("bd_rf", [P, P], MM_DT, kind="ExternalInput")
    bd_rh = nc.dram_tensor("bd_rh", [P, P], MM_DT, kind="ExternalInput")
    ident = nc.dram_tensor("ident", [P, P], MM_DT, kind="ExternalInput")
    m2 = nc.dram_tensor("m2", [P + 1, G], F16, kind="ExternalInput")
    fcw125 = nc.dram_tensor("fcw125", [P, 4], F16, kind="ExternalInput")
    fcb = nc.dram_tensor("fcb", [1, 4], F16, kind="ExternalInput")
    zer = nc.dram_tensor("zer", [45, 2 * T], F16, kind="ExternalInput")
    out = nc.dram_tensor("out", [BC, 4], F32, kind="ExternalOutput")
    dbg = os.environ.get("MGU_DEBUG_DUMP", "0") == "1"
    if dbg:
        p12d = [nc.dram_tensor(f"p12d_{b}", [P, 2 * T], F16,
                               kind="ExternalOutput") for b in range(BL)]
        hbd = [nc.dram_tensor(f"hbd_{b}", [P, T + 64], F16,
                              kind="ExternalOutput") for b in range(BL)]

    with tile.TileContext(nc) as tc:
        with (
            tc.tile_pool(name="consts", bufs=1) as consts,
            tc.tile_pool(name="master", bufs=1) as master,
            tc.tile_pool(name="xt", bufs=16) as xt_pool,
            tc.tile_pool(name="stg", bufs=6) as stg_pool,
            tc.tile_pool(name="ps1", bufs=2, space="PSUM") as ps1_pool,
            tc.tile_pool(name="ps2", bufs=3, space="PSUM") as ps2_pool,
            tc.tile_pool(name="gv1", bufs=3) as gv1_pool,
            tc.tile_pool(name="gw", bufs=3) as gw_pool,
            tc.tile_pool(name="ghv", bufs=3) as ghv_pool,
            tc.tile_pool(name="gv2", bufs=3) as gv2_pool,
            tc.tile_pool(name="gm", bufs=3) as gm_pool,
            tc.tile_pool(name="head", bufs=1) as head_pool,
        ):
            # ---- persistent master-layout tensors (allocated first so
            # the gpsimd dead-lane memsets can precede the const DMAs) ----
            P12 = [master.tile([P, 2 * T], F16, tag=f"P12_{b}", name=f"P12_{b}")
                   for b in range(BL)]
            Hb = [master.tile([P, T + 64], F16, tag=f"Hb_{b}", name=f"Hb_{b}")
                  for b in range(BL)]
            for b in range(BL):
                nc.vector.memset(Hb[b][:, 0:1], 0.0)   # h0 = 0

            # ---- constants ----
            # khp/b128 (needed by the first projections) + the P12
            # dead-lane zeros ride gpsimd; the late-needed sweep weights
            # ride the scalar ring ahead of the remaps. The sync ring is
            # dedicated to the 16 tx loads (a remap interleaved with loads
            # head-of-line blocks the ring on its drain semaphore).
            khp_sb = consts.tile([2 * D, 32], F16, tag="khp")
            b128_sb = consts.tile([128, 1], F32, tag="b128")
            bdrf_sb = consts.tile([P, P], MM_DT, tag="bdrf")
            bdrh_sb = consts.tile([P, P], MM_DT, tag="bdrh")
            id_sb = consts.tile([P, P], MM_DT, tag="ident")
            m2_sb = consts.tile([P + 1, G], F16, tag="m2")
            fcw_sb = consts.tile([P, 4], F16, tag="fcw125")
            nc.gpsimd.dma_start(khp_sb[:], khp[:])
            nc.gpsimd.dma_start(b128_sb[:], b128[:])
            # dead lanes (g >= NB[bl]) must be ZERO: the block-diag matmuls
            # multiply every lane by the weight column (0 * NaN = NaN would
            # pollute live psum rows). DMA-zeroed (engine memsets cost
            # ~5.4us of early DVE time; DMA partition bases are free).
            nc.gpsimd.dma_start(P12[0][5 * NB[0]:P, :], zer[0:P - 5 * NB[0]])
            nc.gpsimd.dma_start(P12[1][5 * NB[1]:P, :], zer[0:P - 5 * NB[1]])
            nc.gpsimd.dma_start(P12[2][5 * NB[2]:P, :], zer[0:P - 5 * NB[2]])
            nc.scalar.dma_start(id_sb[:], ident[:])
            nc.scalar.dma_start(bdrf_sb[:], bd_rf[:])
            nc.scalar.dma_start(bdrh_sb[:], bd_rh[:])
            nc.scalar.dma_start(m2_sb[:], m2[:])
            nc.scalar.dma_start(fcw_sb[:], fcw125[:])
            # head rhs tiles: rows 0..124 written per block at the final
            # sweep; row 125 = fc_b (ones row of m2 adds the bias)
            rhs2 = [head_pool.tile([P + 1, 4], F16, tag=f"rhs2_{b}",
                                   name=f"rhs2_{b}") for b in range(BL)]
            for b in range(BL):
                nc.scalar.dma_start(rhs2[b][P:P + 1, :], fcb[:])
            # preload the Exp ACT table now (1.28us); otherwise it loads
            # lazily right before the head's exp, on the tail critical path
            exd = head_pool.tile([1, 1], F32, tag="exd")
            nc.scalar.activation(exd[:], b128_sb[0:1, 0:1], AF.Exp)

            # ---- Phase 1: plain transposed loads + projection ----
            # Loads are emitted per block (see the emission loop below):
            # the DMA engines are a single globally-serialized resource
            # (~650ns issue + bytes/360GBps per instruction), so block 0's
            # remaps must not queue behind later blocks' loads.
            xt2s = {}

            def emit_loads(bl):
                for qq in range(*([0, 6], [6, 12], [12, 16])[bl]):
                    xt = xt_pool.tile([2 * D, 2 * T], F16, tag="xt",
                                      name="xt")
                    nc.sync.dma_start(out=xt[:], in_=txpt[qq])
                    xt2s[qq] = xt

            def xt_slice(q, th):
                return xt2s[q // 2][:, (q % 2) * T + th * 512:
                                    (q % 2) * T + th * 512 + 512]

            remap_cnt = [0]

            def emit_group(grp):
                bl = GRP_BL[grp]
                g0 = GRP_G0[grp]
                np_ = GRP_NP[grp]
                q0 = 3 * grp
                stg = stg_pool.tile([128, 2 * 512], F16, tag="stg")
                for th in range(2):
                    ps = ps1_pool.tile([128, 512], F32, tag="psA")
                    for ql in range(np_):
                        nc.tensor.matmul(
                            ps[32 * ql:32 * ql + 32, :],
                            lhsT=khp_sb[:],
                            rhs=xt_slice(q0 + ql, th),
                            start=True, stop=True,
                        )
                    nrow = 32 * np_
                    # drains on DVE: keeps the scalar queue free for the
                    # sweep activations
                    nc.vector.tensor_scalar(
                        stg[:nrow, 512 * th:512 * th + 512], ps[:nrow, :],
                        b128_sb[:nrow, :], None, ALU.add)
                # remap (both th halves merged -> 33 DMAs total; each DMA
                # instruction costs ~650-784ns of serial ring issue):
                # src rows 32*ql + 2*(5*b01+u) + gate (contig 20), free
                # (th, t) -> P12[bl] partition 5*(g0 + 2*ql + b01) + u,
                # free col gate*1024 + th*512 + t.
                for ql in range(np_):
                    s_ap = stg[32 * ql:32 * ql + 20, :]
                    d_ap = (P12[bl][5 * (g0 + 2 * ql):
                                    5 * (g0 + 2 * ql) + 10, :]
                            .rearrange("p (gate tt t) -> p gate tt t",
                                       gate=2, tt=2))
                    eng = (nc.scalar, nc.gpsimd)[remap_cnt[0] % 2]
                    remap_cnt[0] += 1
                    eng.dma_start(out=d_ap, in_=s_ap)

            # ---- Phase 2 helpers ----
            def mm_pair(ps_t, w_sb, rhs_full, start):
                # accumulate w_sb.T @ rhs into ps_t ([P, T]); 512-col halves
                # (a single 1024-col matmul crosses a psum bank -> illegal)
                for c in range(2):
                    sl = slice(c * 512, (c + 1) * 512)
                    nc.tensor.matmul(ps_t[:, sl], lhsT=w_sb[:],
                                     rhs=rhs_full[:, sl],
                                     start=start, stop=not start)

            def scan_block(bl, w, m):
                # h[t] = w[t]*h[t-1] + m[t], fp32 state (DVE only).
                nc.vector.tensor_tensor_scan(
                    Hb[bl][:, 1:513], w[:, 0:512], m[:, 0:512],
                    0.0, ALU.mult, ALU.add)
                nc.vector.tensor_tensor_scan(
                    Hb[bl][:, 513:T + 1], w[:, 512:T], m[:, 512:T],
                    Hb[bl][:, 512:513], ALU.mult, ALU.add)

            def emit_head(bl):
                # logits = M2.T @ (fcw125 * h_T ++ fc_b); per-block head +
                # softmax + out DMA so block 0's output leaves while block
                # 2 is still scanning (the shared tail measured ~7us).
                # (tensor_scalar wants an f32 scalar AP -> cast h_T col)
                hcol = head_pool.tile([P, 1], F32, tag=f"hcol_{bl}",
                                      name=f"hcol_{bl}")
                nc.vector.tensor_scalar(hcol[:], Hb[bl][:, T:T + 1],
                                        1.0, None, ALU.mult)
                nc.vector.tensor_scalar(rhs2[bl][0:P, :], fcw_sb[:],
                                        hcol[:], None, ALU.mult)
                pl = ps1_pool.tile([G, 4], F32, tag="psA", name=f"pl_{bl}")
                nc.tensor.matmul(pl[:], lhsT=m2_sb[:], rhs=rhs2[bl][:],
                                 start=True, stop=True)
                # |logits| < ~3: exp cannot overflow f32 -> skip max-shift
                ex = head_pool.tile([G, 4], F32, tag=f"ex_{bl}",
                                    name=f"ex_{bl}")
                sm = head_pool.tile([G, 1], F32, tag=f"sm_{bl}",
                                    name=f"sm_{bl}")
                nc.scalar.activation(ex[:], pl[:], AF.Exp, accum_out=sm[:])
                ri = head_pool.tile([G, 1], F32, tag=f"ri_{bl}",
                                    name=f"ri_{bl}")
                nc.vector.reciprocal(ri[:], sm[:])
                op = head_pool.tile([G, 4], F32, tag=f"op_{bl}",
                                    name=f"op_{bl}")
                nc.vector.tensor_scalar(op[:], ex[:], ri[:], None, ALU.mult)
                eng = (nc.sync, nc.scalar, nc.gpsimd)[bl]
                eng.dma_start(out=out[B0[bl]:B0[bl] + NB[bl], :],
                              in_=op[0:NB[bl], :])

            def emit_sweep0(bl):
                # sweep 0: h == 0 -> pa = P1, pb = P2, no matmuls
                v1 = gv1_pool.tile([P, T], F16, tag="v1", name="v1")
                nc.scalar.activation(v1[:], P12[bl][:, 0:T], AF.Sigmoid)
                v2 = gv2_pool.tile([P, T], F16, tag="v2", name="v2")
                nc.scalar.activation(v2[:], P12[bl][:, T:2 * T], AF.Tanh)
                w = gw_pool.tile([P, T], F16, tag="w", name="w")
                nc.scalar.activation(w[:], v1[:], AF.Copy,
                                     bias=1.0, scale=-1.0)
                m = gm_pool.tile([P, T], F16, tag="m", name="m")
                nc.vector.tensor_tensor(m[:], v1[:], v2[:], ALU.mult)
                scan_block(bl, w, m)

            # ---- emission ----
            # Block-major phase 1 + sweep 0: block bl's sweep-0 unit starts
            # as soon as its last remap lands, overlapping later blocks'
            # loads/projections.
            for bl in range(BL):
                emit_loads(bl)
                for grp in BL_GRPS[bl]:
                    emit_group(grp)
                emit_sweep0(bl)

            # Lockstep sweeps with cross-block weight batching (PE
            # pipelining; per-unit emission measured 60us slower on v2).
            for s in range(1, NSWEEPS):
                # per-block MM interleave: pa[bl] completes after ITS 4
                # MMs instead of waiting the whole cross-block batch (the
                # batched order made pb[0] transitively wait on hv[2],
                # stretching the sweep cadence to ~15.5us vs ~11us busy)
                pa = [ps2_pool.tile([P, T], F32, tag="ps2", name="pa")
                      for _ in range(BL)]
                for bl in range(BL):
                    mm_pair(pa[bl], bdrf_sb, Hb[bl][:, 0:T], start=True)
                    mm_pair(pa[bl], id_sb, P12[bl][:, 0:T], start=False)
                v1s, ws, hvs = [], [], []
                for bl in range(BL):
                    v1 = gv1_pool.tile([P, T], F16, tag="v1", name="v1")
                    nc.scalar.activation(v1[:], pa[bl][:], AF.Sigmoid)
                    v1s.append(v1)
                    hv = ghv_pool.tile([P, T], F16, tag="hv", name="hv")
                    nc.vector.tensor_tensor(hv[:], Hb[bl][:, 0:T], v1[:],
                                            ALU.mult)
                    hvs.append(hv)
                pb = [ps2_pool.tile([P, T], F32, tag="ps2", name="pb")
                      for _ in range(BL)]
                for bl in range(BL):
                    mm_pair(pb[bl], bdrh_sb, hvs[bl][:], start=True)
                    mm_pair(pb[bl], id_sb, P12[bl][:, T:2 * T], start=False)
                for bl in range(BL):
                    v2 = gv2_pool.tile([P, T], F16, tag="v2", name="v2")
                    nc.scalar.activation(v2[:], pb[bl][:], AF.Tanh)
                    # w = 1 - v1 as an ACT Copy (cannot read pa here: its
                    # psum buffer is already recycled into pb). On ACT
                    # right after the same block's tanh: off the critical
                    # chain (only the scan reads it); gpsimd measured
                    # worse (shared SBUF port contention slowed DVE ~35%)
                    w = gw_pool.tile([P, T], F16, tag="w", name="w")
                    nc.scalar.activation(w[:], v1s[bl][:], AF.Copy,
                                         bias=1.0, scale=-1.0)
                    ws.append(w)
                    m = gm_pool.tile([P, T], F16, tag="m", name="m")
                    nc.vector.tensor_tensor(m[:], v1s[bl][:], v2[:],
                                            ALU.mult)
                    scan_block(bl, ws[bl], m)
                    if s == NSWEEPS - 1:
                        emit_head(bl)

            if dbg:
                for b in range(BL):
                    nc.gpsimd.dma_start(out=p12d[b][:], in_=P12[b][:])
                    nc.gpsimd.dma_start(out=hbd[b][:], in_=Hb[b][:])

    nc.compile()
    return nc


def _prep_host_inputs(kernel, rec_kernel, bias, fc_w, fc_b):
    f32 = np.float32
    k = np.asarray(kernel, f32).astype(np.float16)    # [64, 10]

    # psum row (within a 32-row pair slot) = 2*(5*b01 + u) + gate
    # (gate innermost so the remap DMA sees one contiguous 20-row run)
    khp = np.zeros((2 * D, 32), np.float16)
    b128 = np.zeros((128, 1), f32)
    bias_f = np.asarray(bias, f32)
    for gate in range(2):
        for b01 in range(2):
            for u in range(U):
                c = 2 * (5 * b01 + u) + gate
                khp[D * b01:D * b01 + D, c] = k[:, 5 * gate + u]
                for ql in range(4):
                    b128[32 * ql + c, 0] = bias_f[5 * gate + u]

    rk = np.asarray(rec_kernel, f32)
    bd_rf = np.zeros((P, P), np.float16)
    bd_rh = np.zeros((P, P), np.float16)
    for g in range(G):
        bd_rf[5 * g:5 * g + 5, 5 * g:5 * g + 5] = rk[:, :U]
        bd_rh[5 * g:5 * g + 5, 5 * g:5 * g + 5] = rk[:, U:]
    ident = np.eye(P, dtype=np.float16)

    # head selector: logits[g, j] = sum_u h[5g+u] fc_w[u, j] + fc_b[j]
    m2 = np.zeros((P + 1, G), np.float16)
    for g in range(G):
        m2[5 * g:5 * g + 5, g] = 1.0
    m2[P, :] = 1.0
    fcw125 = np.tile(np.asarray(fc_w, f32), (G, 1)).astype(np.float16)
    fcb = np.asarray(fc_b, f32).reshape(1, 4).astype(np.float16)
    zer = np.zeros((45, 2 * T), np.float16)
    return dict(khp=khp, b128=b128, bd_rf=bd_rf, bd_rh=bd_rh, ident=ident,
                m2=m2, fcw125=fcw125, fcb=fcb, zer=zer)


_CACHE = {}


def kernel(tx, kernel, rec_kernel, bias, fc_w, fc_b, _want_time=False):
    tx = np.asarray(tx, np.float32)
    host = _prep_host_inputs(kernel, rec_kernel, bias, fc_w, fc_b)

    # fp16 pre-transposed tx: [core, load, (b01, d), (q_lo, t)]
    # load qq covers pairs 2qq, 2qq+1; pair pq covers batches 2pq, 2pq+1.
    txpt_all = np.ascontiguousarray(
        tx.reshape(NCORES, NLOAD, 2, 2, T, D)    # c, qq, q_lo, b01, t, d
        .transpose(0, 1, 3, 5, 2, 4)             # c, qq, b01, d, q_lo, t
        .reshape(NCORES, NLOAD, 2 * D, 2 * T).astype(np.float16))

    if "nc" not in _CACHE:
        _CACHE["nc"] = build_program()
    nc = _CACHE["nc"]

    in_maps = []
    for c in range(NCORES):
        m = {"txpt": txpt_all[c]}
        m.update(host)
        in_maps.append(m)

    try:
        res = run_bass_kernel_spmd(
            nc, in_maps, core_ids=list(range(NCORES)), trace=_want_time
        )
    except ModuleNotFoundError:
        res = run_bass_kernel_spmd(
            nc, in_maps, core_ids=list(range(NCORES)), trace=False
        )
    outs = [res.results[c]["out"] for c in range(NCORES)]
    full = np.concatenate(outs, axis=0)
    if _want_time:
        _CACHE["res"] = res
        return full, res.exec_time_ns
    return full


# revision 36
# speedup vs baseline: 1.2015x; 1.0822x over previous
"""MGU RNN (nn_Network_82394652607110) — Trainium2 Bass kernel, v4.

v3 (176935 ns) -> v4 changes, from trace analysis:
 - Host pre-transposes tx, so phase 1 loads are plain contiguous
   [128, 2048] DMAs spread across BOTH HWDGE rings instead of 16
   serialized xbar transposes (xbar transposes occupy the issuing
   engine for the full transfer: 2.07us each, one ring only -> 33us
   of Scalar-engine time + a WAR cascade that stretched phase 1 to
   80us and delayed sweep-0's sigmoid to 78us).
 - Block-major emission: each block's groups are followed by its
   sweep-0 unit, so the sweep pipeline starts as soon as block 0's
   P12 lands (~12us) and overlaps the rest of phase 1.
 - w = 1 - sigmoid(pa) computed as sigmoid(-pa) on ACT (activation
   scale=-1), moving ~2.7us/sweep off the Vector engine (the phase-2
   bottleneck at ~80% busy).
 - Head reworked: logits_g = sum_u h_T[g,u] fc_w[u,:] + fc_b computed
   as a selector matmul (lhsT = M2 [126, 25] with a ones bias row,
   rhs = fcw125 * Hb[:, T] built by one DVE op per block), replacing
   15 tiny partition-strided gather DMAs (~5us tail).
 - Memsets trimmed to P12 dead lanes (32-aligned bases) + Hb col 0.

Kept from v3 (measured hazards -- avoid regressing!):
 - DMA APs with >=2 partition dims mis-lower -> remaps stay one
   contiguous 20-row partition run; engine ops need 32-aligned
   partition bases; matmul psum base must be 0/32/64; gpsimd cannot
   read PSUM; gpsimd has no scan.
 - Quasi-DEER: NSWEEPS=6 (deterministic max err 9.5e-3, L2 6.9e-4 vs
   the 2e-2 gate). Sweep 0 specializes h=0. Matmuls batched per
   weight ACROSS blocks; psum drains for phase 1 on DVE; scans in
   2x512 chunks (a single 1024-col scan runs at 4 cyc/col vs 2.5).

Layout: per block bl in {0,1,2}: P12[bl] [125, 2048] fp16 (p1 cols
0..1024, p2 cols 1024..2048), partitions 5g+u, batch b = B0[bl]+g,
live groups 24/24/16 of 25. Hb[bl] [125, 1088] fp16: col 0 = zero
initial state, scan writes 1..1024.
"""

import os
import numpy as np

import concourse.bass as bass
import concourse.bacc as bacc
import concourse.tile as tile
import concourse.mybir as mybir
from concourse.bass_utils import run_bass_kernel_spmd

dt = mybir.dt
AF = mybir.ActivationFunctionType
ALU = mybir.AluOpType

# Problem constants (hardcoded per harness contract)
U = 5
T = 1024
D = 64
B = 512
NCORES = 8
BC = B // NCORES          # 64 batch per core
NPAIR = BC // 2           # 32
NLOAD = NPAIR // 2        # 16 loads, two pairs each

G = 25                    # partition groups per block
P = G * U                 # 125 partitions
BL = 3                    # blocks
B0 = [0, 24, 48]          # first batch of each block
NB = [24, 24, 16]         # live batches (groups) per block
# 6-batch psum groups (3 pairs at bases 0/32/64); last group has 2 pairs
GRP_BL = [0, 0, 0, 0, 1, 1, 1, 1, 2, 2, 2]
GRP_G0 = [0, 6, 12, 18, 0, 6, 12, 18, 0, 6, 12]
GRP_NP = [3, 3, 3, 3, 3, 3, 3, 3, 3, 3, 2]   # pairs per group
BL_GRPS = [[0, 1, 2, 3], [4, 5, 6, 7], [8, 9, 10]]

NSWEEPS = int(os.environ.get("MGU_NSWEEPS", "6"))
MM_DT = dt.float16
F16 = dt.float16
F32 = dt.float32


def build_program():
    nc = bacc.Bacc("TRN2", target_bir_lowering=False, debug=False)

    # pre-transposed tx: [load, (b01 d), (q_lo t)]
    txpt = nc.dram_tensor("txpt", [NLOAD, 2 * D, 2 * T], F16,
                          kind="ExternalInput")
    # khp3[ql]: projection weights for pair ql of a group, zero-padded so
    # the three accumulating matmuls write psum rows 20*ql..20*ql+20 of
    # ONE compact [60, 512] region (out partition = lhsT column; psum
    # write base stays 0) -> one remap DMA per group instead of three
    khp3 = nc.dram_tensor("khp3", [3, 2 * D, 60], F16, kind="ExternalInput")
    b60 = nc.dram_tensor("b60", [60, 1], F32, kind="ExternalInput")
    bd_rf = nc.dram_tensor("bd_rf", [P, P], MM_DT, kind="ExternalInput")
    bd_rh = nc.dram_tensor("bd_rh", [P, P], MM_DT, kind="ExternalInput")
    ident = nc.dram_tensor("ident", [P, P], MM_DT, kind="ExternalInput")
    m2 = nc.dram_tensor("m2", [P + 1, G], F16, kind="ExternalInput")
    fcw125 = nc.dram_tensor("fcw125", [P, 4], F16, kind="ExternalInput")
    fcb = nc.dram_tensor("fcb", [1, 4], F16, kind="ExternalInput")
    zer = nc.dram_tensor("zer", [45, 2 * T], F16, kind="ExternalInput")
    out = nc.dram_tensor("out", [BC, 4], F32, kind="ExternalOutput")
    dbg = os.environ.get("MGU_DEBUG_DUMP", "0") == "1"
    if dbg:
        p12d = [nc.dram_tensor(f"p12d_{b}", [P, 2 * T], F16,
                               kind="ExternalOutput") for b in range(BL)]
        hbd = [nc.dram_tensor(f"hbd_{b}", [P, T + 64], F16,
                              kind="ExternalOutput") for b in range(BL)]

    with tile.TileContext(nc) as tc:
        with (
            tc.tile_pool(name="consts", bufs=1) as consts,
            tc.tile_pool(name="master", bufs=1) as master,
            tc.tile_pool(name="xt", bufs=16) as xt_pool,
            tc.tile_pool(name="stg", bufs=6) as stg_pool,
            tc.tile_pool(name="ps1", bufs=2, space="PSUM") as ps1_pool,
            tc.tile_pool(name="ps2", bufs=3, space="PSUM") as ps2_pool,
            tc.tile_pool(name="gv1", bufs=3) as gv1_pool,
            tc.tile_pool(name="gw", bufs=3) as gw_pool,
            tc.tile_pool(name="ghv", bufs=3) as ghv_pool,
            tc.tile_pool(name="gv2", bufs=3) as gv2_pool,
            tc.tile_pool(name="gm", bufs=3) as gm_pool,
            tc.tile_pool(name="head", bufs=1) as head_pool,
        ):
            # ---- persistent master-layout tensors (allocated first so
            # the gpsimd dead-lane memsets can precede the const DMAs) ----
            P12 = [master.tile([P, 2 * T], F16, tag=f"P12_{b}", name=f"P12_{b}")
                   for b in range(BL)]
            Hb = [master.tile([P, T + 64], F16, tag=f"Hb_{b}", name=f"Hb_{b}")
                  for b in range(BL)]
            for b in range(BL):
                nc.vector.memset(Hb[b][:, 0:1], 0.0)   # h0 = 0

            # ---- constants ----
            # khp/b128 (needed by the first projections) + the P12
            # dead-lane zeros ride gpsimd; the late-needed sweep weights
            # ride the scalar ring ahead of the remaps. The sync ring is
            # dedicated to the 16 tx loads (a remap interleaved with loads
            # head-of-line blocks the ring on its drain semaphore).
            khp_sb = [consts.tile([2 * D, 60], F16, tag=f"khp3_{q}",
                                  name=f"khp3_{q}")
                      for q in range(3)]
            b60_sb = consts.tile([60, 1], F32, tag="b60")
            bdrf_sb = consts.tile([P, P], MM_DT, tag="bdrf")
            bdrh_sb = consts.tile([P, P], MM_DT, tag="bdrh")
            id_sb = consts.tile([P, P], MM_DT, tag="ident")
            m2_sb = consts.tile([P + 1, G], F16, tag="m2")
            fcw_sb = consts.tile([P, 4], F16, tag="fcw125")
            for q in range(3):
                nc.gpsimd.dma_start(khp_sb[q][:], khp3[q])
            nc.gpsimd.dma_start(b60_sb[:], b60[:])
            # dead lanes (g >= NB[bl]) must be ZERO: the block-diag matmuls
            # multiply every lane by the weight column (0 * NaN = NaN would
            # pollute live psum rows). DMA-zeroed (engine memsets cost
            # ~5.4us of early DVE time; DMA partition bases are free).
            nc.gpsimd.dma_start(P12[0][5 * NB[0]:P, :], zer[0:P - 5 * NB[0]])
            nc.gpsimd.dma_start(P12[1][5 * NB[1]:P, :], zer[0:P - 5 * NB[1]])
            nc.gpsimd.dma_start(P12[2][5 * NB[2]:P, :], zer[0:P - 5 * NB[2]])
            # late-needed sweep weights ride the sync ring AFTER the loads
            # (sync is otherwise idle then; the scalar ring must stay empty
            # so block 0's remaps and the sweep ACT ops issue promptly)
            def emit_late_consts():
                nc.sync.dma_start(id_sb[:], ident[:])
                nc.sync.dma_start(bdrf_sb[:], bd_rf[:])
                nc.sync.dma_start(bdrh_sb[:], bd_rh[:])
                nc.sync.dma_start(m2_sb[:], m2[:])
                nc.sync.dma_start(fcw_sb[:], fcw125[:])
                for b in range(BL):
                    nc.sync.dma_start(rhs2[b][P:P + 1, :], fcb[:])
            # head rhs tiles: rows 0..124 written per block at the final
            # sweep; row 125 = fc_b (ones row of m2 adds the bias)
            rhs2 = [head_pool.tile([P + 1, 4], F16, tag=f"rhs2_{b}",
                                   name=f"rhs2_{b}") for b in range(BL)]
            # preload the Exp ACT table now (1.28us); otherwise it loads
            # lazily right before the head's exp, on the tail critical path
            exd = head_pool.tile([1, 1], F32, tag="exd")
            nc.scalar.activation(exd[:], b60_sb[0:1, 0:1], AF.Exp)

            # ---- Phase 1: plain transposed loads + projection ----
            # Loads are emitted per block (see the emission loop below):
            # the DMA engines are a single globally-serialized resource
            # (~650ns issue + bytes/360GBps per instruction), so block 0's
            # remaps must not queue behind later blocks' loads.
            xt2s = {}

            def emit_loads(bl):
                for qq in range(*([0, 6], [6, 12], [12, 16])[bl]):
                    xt = xt_pool.tile([2 * D, 2 * T], F16, tag="xt",
                                      name="xt")
                    nc.sync.dma_start(out=xt[:], in_=txpt[qq])
                    xt2s[qq] = xt

            def xt_slice(q, th):
                return xt2s[q // 2][:, (q % 2) * T + th * 512:
                                    (q % 2) * T + th * 512 + 512]

            remap_cnt = [0]

            def emit_group(grp):
                bl = GRP_BL[grp]
                g0 = GRP_G0[grp]
                np_ = GRP_NP[grp]
                q0 = 3 * grp
                nrow = 20 * np_
                stg = stg_pool.tile([60, 2 * 512], F16, tag="stg")
                for th in range(2):
                    ps = ps1_pool.tile([60, 512], F32, tag="psA")
                    # the 3 pairs ACCUMULATE into one compact [60, 512]
                    # region: khp3[ql] is zero outside cols 20ql..20ql+20
                    for ql in range(np_):
                        nc.tensor.matmul(
                            ps[:nrow, :],
                            lhsT=khp_sb[ql][:, :nrow],
                            rhs=xt_slice(q0 + ql, th),
                            start=(ql == 0), stop=(ql == np_ - 1),
                        )
                    # drains on DVE: keeps the scalar queue free for the
                    # sweep activations
                    nc.vector.tensor_scalar(
                        stg[:nrow, 512 * th:512 * th + 512], ps[:nrow, :],
                        b60_sb[:nrow, :], None, ALU.add)
                # ONE remap per group (11 total; each DMA instruction costs
                # ~650-784ns serial ring issue + ~0.7us transfer):
                # src row 20*ql + 2*(5*b01+u) + gate, free (th, t) ->
                # P12[bl] partition 5*(g0 + 2*ql + b01) + u,
                # free col gate*1024 + th*512 + t.
                s_ap = stg[:nrow, :]
                d_ap = (P12[bl][5 * g0:5 * g0 + 10 * np_, :]
                        .rearrange("p (gate tt t) -> p gate tt t",
                                   gate=2, tt=2))
                # block 0's remaps on the otherwise-empty scalar ring so
                # nothing delays them; later blocks' on gpsimd
                eng = nc.scalar if bl == 0 else nc.gpsimd
                remap_cnt[0] += 1
                eng.dma_start(out=d_ap, in_=s_ap)

            # ---- Phase 2 helpers ----
            def mm_pair(ps_t, w_sb, rhs_full, start):
                # accumulate w_sb.T @ rhs into ps_t ([P, T]); 512-col halves
                # (a single 1024-col matmul crosses a psum bank -> illegal)
                for c in range(2):
                    sl = slice(c * 512, (c + 1) * 512)
                    nc.tensor.matmul(ps_t[:, sl], lhsT=w_sb[:],
                                     rhs=rhs_full[:, sl],
                                     start=start, stop=not start)

            def scan_block(bl, w, m):
                # h[t] = w[t]*h[t-1] + m[t], fp32 state (DVE only).
                nc.vector.tensor_tensor_scan(
                    Hb[bl][:, 1:513], w[:, 0:512], m[:, 0:512],
                    0.0, ALU.mult, ALU.add)
                nc.vector.tensor_tensor_scan(
                    Hb[bl][:, 513:T + 1], w[:, 512:T], m[:, 512:T],
                    Hb[bl][:, 512:513], ALU.mult, ALU.add)

            def emit_head(bl):
                # logits = M2.T @ (fcw125 * h_T ++ fc_b); per-block head +
                # softmax + out DMA so block 0's output leaves while block
                # 2 is still scanning (the shared tail measured ~7us).
                # (tensor_scalar wants an f32 scalar AP -> cast h_T col)
                hcol = head_pool.tile([P, 1], F32, tag=f"hcol_{bl}",
                                      name=f"hcol_{bl}")
                nc.vector.tensor_scalar(hcol[:], Hb[bl][:, T:T + 1],
                                        1.0, None, ALU.mult)
                nc.vector.tensor_scalar(rhs2[bl][0:P, :], fcw_sb[:],
                                        hcol[:], None, ALU.mult)
                pl = ps1_pool.tile([G, 4], F32, tag="psA", name=f"pl_{bl}")
                nc.tensor.matmul(pl[:], lhsT=m2_sb[:], rhs=rhs2[bl][:],
                                 start=True, stop=True)
                # |logits| < ~3: exp cannot overflow f32 -> skip max-shift
                ex = head_pool.tile([G, 4], F32, tag=f"ex_{bl}",
                                    name=f"ex_{bl}")
                sm = head_pool.tile([G, 1], F32, tag=f"sm_{bl}",
                                    name=f"sm_{bl}")
                nc.scalar.activation(ex[:], pl[:], AF.Exp, accum_out=sm[:])
                ri = head_pool.tile([G, 1], F32, tag=f"ri_{bl}",
                                    name=f"ri_{bl}")
                nc.vector.reciprocal(ri[:], sm[:])
                op = head_pool.tile([G, 4], F32, tag=f"op_{bl}",
                                    name=f"op_{bl}")
                nc.vector.tensor_scalar(op[:], ex[:], ri[:], None, ALU.mult)
                eng = (nc.sync, nc.scalar, nc.gpsimd)[bl]
                eng.dma_start(out=out[B0[bl]:B0[bl] + NB[bl], :],
                              in_=op[0:NB[bl], :])

            def emit_sweep0(bl):
                # sweep 0: h == 0 -> pa = P1, pb = P2, no matmuls
                v1 = gv1_pool.tile([P, T], F16, tag="v1", name="v1")
                nc.scalar.activation(v1[:], P12[bl][:, 0:T], AF.Sigmoid)
                v2 = gv2_pool.tile([P, T], F16, tag="v2", name="v2")
                nc.scalar.activation(v2[:], P12[bl][:, T:2 * T], AF.Tanh)
                w = gw_pool.tile([P, T], F16, tag="w", name="w")
                nc.scalar.activation(w[:], v1[:], AF.Copy,
                                     bias=1.0, scale=-1.0)
                m = gm_pool.tile([P, T], F16, tag="m", name="m")
                nc.vector.tensor_tensor(m[:], v1[:], v2[:], ALU.mult)
                scan_block(bl, w, m)

            # ---- emission ----
            # Block-major phase 1 + sweep 0: block bl's sweep-0 unit starts
            # as soon as its last remap lands, overlapping later blocks'
            # loads/projections.
            for bl in range(BL):
                emit_loads(bl)
                if bl == 0:
                    # sweep weights slot in behind block 0's loads: tiny
                    # transfers, and the sync issue delay for later loads
                    # hides behind the serialized DMA device anyway
                    emit_late_consts()
                for grp in BL_GRPS[bl]:
                    emit_group(grp)
                emit_sweep0(bl)

            # Lockstep sweeps with cross-block weight batching (PE
            # pipelining; per-unit emission measured 60us slower on v2).
            for s in range(1, NSWEEPS):
                # per-block MM interleave: pa[bl] completes after ITS 4
                # MMs instead of waiting the whole cross-block batch (the
                # batched order made pb[0] transitively wait on hv[2],
                # stretching the sweep cadence to ~15.5us vs ~11us busy)
                pa = [ps2_pool.tile([P, T], F32, tag="ps2", name="pa")
                      for _ in range(BL)]
                for bl in range(BL):
                    mm_pair(pa[bl], bdrf_sb, Hb[bl][:, 0:T], start=True)
                    mm_pair(pa[bl], id_sb, P12[bl][:, 0:T], start=False)
                v1s, ws, hvs = [], [], []
                for bl in range(BL):
                    v1 = gv1_pool.tile([P, T], F16, tag="v1", name="v1")
                    nc.scalar.activation(v1[:], pa[bl][:], AF.Sigmoid)
                    v1s.append(v1)
                    hv = ghv_pool.tile([P, T], F16, tag="hv", name="hv")
                    nc.vector.tensor_tensor(hv[:], Hb[bl][:, 0:T], v1[:],
                                            ALU.mult)
                    hvs.append(hv)
                pb = [ps2_pool.tile([P, T], F32, tag="ps2", name="pb")
                      for _ in range(BL)]
                for bl in range(BL):
                    mm_pair(pb[bl], bdrh_sb, hvs[bl][:], start=True)
                    mm_pair(pb[bl], id_sb, P12[bl][:, T:2 * T], start=False)
                for bl in range(BL):
                    v2 = gv2_pool.tile([P, T], F16, tag="v2", name="v2")
                    nc.scalar.activation(v2[:], pb[bl][:], AF.Tanh)
                    # w = 1 - v1 as an ACT Copy (cannot read pa here: its
                    # psum buffer is already recycled into pb). On ACT
                    # right after the same block's tanh: off the critical
                    # chain (only the scan reads it); gpsimd measured
                    # worse (shared SBUF port contention slowed DVE ~35%)
                    w = gw_pool.tile([P, T], F16, tag="w", name="w")
                    nc.scalar.activation(w[:], v1s[bl][:], AF.Copy,
                                         bias=1.0, scale=-1.0)
                    ws.append(w)
                    m = gm_pool.tile([P, T], F16, tag="m", name="m")
                    nc.vector.tensor_tensor(m[:], v1s[bl][:], v2[:],
                                            ALU.mult)
                    scan_block(bl, ws[bl], m)
                    if s == NSWEEPS - 1:
                        emit_head(bl)

            if dbg:
                for b in range(BL):
                    nc.gpsimd.dma_start(out=p12d[b][:], in_=P12[b][:])
                    nc.gpsimd.dma_start(out=hbd[b][:], in_=Hb[b][:])

    nc.compile()
    return nc


def _prep_host_inputs(kernel, rec_kernel, bias, fc_w, fc_b):
    f32 = np.float32
    k = np.asarray(kernel, f32).astype(np.float16)    # [64, 10]

    # compact psum row = 20*ql + 2*(5*b01 + u) + gate (gate innermost so
    # the remap DMA sees contiguous gate pairs); khp3[ql] zero-padded so
    # the three pair-matmuls accumulate into one [60, 512] psum region
    khp3 = np.zeros((3, 2 * D, 60), np.float16)
    b60 = np.zeros((60, 1), f32)
    bias_f = np.asarray(bias, f32)
    for gate in range(2):
        for b01 in range(2):
            for u in range(U):
                c = 2 * (5 * b01 + u) + gate
                for ql in range(3):
                    khp3[ql, D * b01:D * b01 + D, 20 * ql + c] = \
                        k[:, 5 * gate + u]
                    b60[20 * ql + c, 0] = bias_f[5 * gate + u]

    rk = np.asarray(rec_kernel, f32)
    bd_rf = np.zeros((P, P), np.float16)
    bd_rh = np.zeros((P, P), np.float16)
    for g in range(G):
        bd_rf[5 * g:5 * g + 5, 5 * g:5 * g + 5] = rk[:, :U]
        bd_rh[5 * g:5 * g + 5, 5 * g:5 * g + 5] = rk[:, U:]
    ident = np.eye(P, dtype=np.float16)

    # head selector: logits[g, j] = sum_u h[5g+u] fc_w[u, j] + fc_b[j]
    m2 = np.zeros((P + 1, G), np.float16)
    for g in range(G):
        m2[5 * g:5 * g + 5, g] = 1.0
    m2[P, :] = 1.0
    fcw125 = np.tile(np.asarray(fc_w, f32), (G, 1)).astype(np.float16)
    fcb = np.asarray(fc_b, f32).reshape(1, 4).astype(np.float16)
    zer = np.zeros((45, 2 * T), np.float16)
    return dict(khp3=khp3, b60=b60, bd_rf=bd_rf, bd_rh=bd_rh, ident=ident,
                m2=m2, fcw125=fcw125, fcb=fcb, zer=zer)


_CACHE = {}


def kernel(tx, kernel, rec_kernel, bias, fc_w, fc_b, _want_time=False):
    tx = np.asarray(tx, np.float32)
    host = _prep_host_inputs(kernel, rec_kernel, bias, fc_w, fc_b)

    # fp16 pre-transposed tx: [core, load, (b01, d), (q_lo, t)]
    # load qq covers pairs 2qq, 2qq+1; pair pq covers batches 2pq, 2pq+1.
    txpt_all = np.ascontiguousarray(
        tx.reshape(NCORES, NLOAD, 2, 2, T, D)    # c, qq, q_lo, b01, t, d
        .transpose(0, 1, 3, 5, 2, 4)             # c, qq, b01, d, q_lo, t
        .reshape(NCORES, NLOAD, 2 * D, 2 * T).astype(np.float16))

    if "nc" not in _CACHE:
        _CACHE["nc"] = build_program()
    nc = _CACHE["nc"]

    in_maps = []
    for c in range(NCORES):
        m = {"txpt": txpt_all[c]}
        m.update(host)
        in_maps.append(m)

    try:
        res = run_bass_kernel_spmd(
            nc, in_maps, core_ids=list(range(NCORES)), trace=_want_time
        )
    except ModuleNotFoundError:
        res = run_bass_kernel_spmd(
            nc, in_maps, core_ids=list(range(NCORES)), trace=False
        )
    outs = [res.results[c]["out"] for c in range(NCORES)]
    full = np.concatenate(outs, axis=0)
    if _want_time:
        _CACHE["res"] = res
        return full, res.exec_time_ns
    return full


# revision 40
# speedup vs baseline: 1.7019x; 1.4165x over previous
"""MGU RNN (nn_Network_82394652607110) — Trainium2 Bass kernel, v4.

v3 (176935 ns) -> v4 changes, from trace analysis:
 - Host pre-transposes tx, so phase 1 loads are plain contiguous
   [128, 2048] DMAs spread across BOTH HWDGE rings instead of 16
   serialized xbar transposes (xbar transposes occupy the issuing
   engine for the full transfer: 2.07us each, one ring only -> 33us
   of Scalar-engine time + a WAR cascade that stretched phase 1 to
   80us and delayed sweep-0's sigmoid to 78us).
 - Block-major emission: each block's groups are followed by its
   sweep-0 unit, so the sweep pipeline starts as soon as block 0's
   P12 lands (~12us) and overlaps the rest of phase 1.
 - w = 1 - sigmoid(pa) computed as sigmoid(-pa) on ACT (activation
   scale=-1), moving ~2.7us/sweep off the Vector engine (the phase-2
   bottleneck at ~80% busy).
 - Head reworked: logits_g = sum_u h_T[g,u] fc_w[u,:] + fc_b computed
   as a selector matmul (lhsT = M2 [126, 25] with a ones bias row,
   rhs = fcw125 * Hb[:, T] built by one DVE op per block), replacing
   15 tiny partition-strided gather DMAs (~5us tail).
 - Memsets trimmed to P12 dead lanes (32-aligned bases) + Hb col 0.

Kept from v3 (measured hazards -- avoid regressing!):
 - DMA APs with >=2 partition dims mis-lower -> remaps stay one
   contiguous 20-row partition run; engine ops need 32-aligned
   partition bases; matmul psum base must be 0/32/64; gpsimd cannot
   read PSUM; gpsimd has no scan.
 - Quasi-DEER: NSWEEPS=6 (deterministic max err 9.5e-3, L2 6.9e-4 vs
   the 2e-2 gate). Sweep 0 specializes h=0. Matmuls batched per
   weight ACROSS blocks; psum drains for phase 1 on DVE; scans in
   2x512 chunks (a single 1024-col scan runs at 4 cyc/col vs 2.5).

Layout: per block bl in {0,1,2}: P12[bl] [125, 2048] fp16 (p1 cols
0..1024, p2 cols 1024..2048), partitions 5g+u, batch b = B0[bl]+g,
live groups 24/24/16 of 25. Hb[bl] [125, 1088] fp16: col 0 = zero
initial state, scan writes 1..1024.
"""

import os
import numpy as np

import concourse.bass as bass
import concourse.bacc as bacc
import concourse.tile as tile
import concourse.mybir as mybir
from concourse.bass_utils import run_bass_kernel_spmd

dt = mybir.dt
AF = mybir.ActivationFunctionType
ALU = mybir.AluOpType

# Problem constants (hardcoded per harness contract)
U = 5
T = 1024
D = 64
B = 512
NCORES = 8
BC = B // NCORES          # 64 batch per core
NPAIR = BC // 2           # 32
NLOAD = NPAIR // 2        # 16 loads, two pairs each

G = 25                    # partition groups per block
P = G * U                 # 125 partitions
BL = 3                    # blocks
B0 = [0, 24, 48]          # first batch of each block
NB = [24, 24, 16]         # live batches (groups) per block
# 6-batch psum groups (3 pairs at bases 0/32/64); last group has 2 pairs
GRP_BL = [0, 0, 0, 0, 1, 1, 1, 1, 2, 2, 2]
GRP_G0 = [0, 6, 12, 18, 0, 6, 12, 18, 0, 6, 12]
GRP_NP = [3, 3, 3, 3, 3, 3, 3, 3, 3, 3, 2]   # pairs per group
BL_GRPS = [[0, 1, 2, 3], [4, 5, 6, 7], [8, 9, 10]]

NSWEEPS = int(os.environ.get("MGU_NSWEEPS", "6"))
# Window start column per sweep (0 = full sweep). A late sweep may run on
# a suffix [W0:T] only: the incoming state error at W0 damps by prod(w)
# over the window, so a suffix refinement approaches full-sweep quality
# at a fraction of the cost. Must be multiples of 256.
W0S = [int(x) for x in os.environ.get(
    "MGU_W0S", ",".join(["0"] * NSWEEPS)).split(",")]
assert len(W0S) == NSWEEPS and W0S[0] == 0
MM_DT = dt.float16
F16 = dt.float16
F32 = dt.float32


def build_program():
    nc = bacc.Bacc("TRN2", target_bir_lowering=False, debug=False)

    # pre-transposed tx: [load, (b01 d), (q_lo t)]
    txpt = nc.dram_tensor("txpt", [NLOAD, 2 * D, 2 * T], F16,
                          kind="ExternalInput")
    # khp3[ql]: projection weights for pair ql of a group, zero-padded so
    # the three accumulating matmuls write psum rows 20*ql..20*ql+20 of
    # ONE compact [60, 512] region (out partition = lhsT column; psum
    # write base stays 0) -> one remap DMA per group instead of three
    khp3 = nc.dram_tensor("khp3", [3, 2 * D, 60], F16, kind="ExternalInput")
    b60 = nc.dram_tensor("b60", [60, 1], F32, kind="ExternalInput")
    bd_rf = nc.dram_tensor("bd_rf", [P, P], MM_DT, kind="ExternalInput")
    bd_rh = nc.dram_tensor("bd_rh", [P, P], MM_DT, kind="ExternalInput")
    ident = nc.dram_tensor("ident", [P, P], MM_DT, kind="ExternalInput")
    m2 = nc.dram_tensor("m2", [P + 1, G], F16, kind="ExternalInput")
    fcw125 = nc.dram_tensor("fcw125", [P, 4], F16, kind="ExternalInput")
    fcb = nc.dram_tensor("fcb", [1, 4], F16, kind="ExternalInput")
    zer = nc.dram_tensor("zer", [45, 2 * T], F16, kind="ExternalInput")
    out = nc.dram_tensor("out", [BC, 4], F32, kind="ExternalOutput")
    dbg = os.environ.get("MGU_DEBUG_DUMP", "0") == "1"
    if dbg:
        p12d = [nc.dram_tensor(f"p12d_{b}", [P, 2 * T], F16,
                               kind="ExternalOutput") for b in range(BL)]
        hbd = [nc.dram_tensor(f"hbd_{b}", [P, T + 64], F16,
                              kind="ExternalOutput") for b in range(BL)]

    with tile.TileContext(nc) as tc:
        with (
            tc.tile_pool(name="consts", bufs=1) as consts,
            tc.tile_pool(name="master", bufs=1) as master,
            tc.tile_pool(name="xt", bufs=16) as xt_pool,
            tc.tile_pool(name="stg", bufs=6) as stg_pool,
            tc.tile_pool(name="ps1", bufs=2, space="PSUM") as ps1_pool,
            tc.tile_pool(name="ps2", bufs=3, space="PSUM") as ps2_pool,
            tc.tile_pool(name="gv1", bufs=3) as gv1_pool,
            tc.tile_pool(name="gw", bufs=3) as gw_pool,
            tc.tile_pool(name="ghv", bufs=3) as ghv_pool,
            tc.tile_pool(name="gv2", bufs=3) as gv2_pool,
            tc.tile_pool(name="gm", bufs=3) as gm_pool,
            tc.tile_pool(name="head", bufs=1) as head_pool,
        ):
            # ---- persistent master-layout tensors (allocated first so
            # the gpsimd dead-lane memsets can precede the const DMAs) ----
            P12 = [master.tile([P, 2 * T], F16, tag=f"P12_{b}", name=f"P12_{b}")
                   for b in range(BL)]
            Hb = [master.tile([P, T + 64], F16, tag=f"Hb_{b}", name=f"Hb_{b}")
                  for b in range(BL)]
            for b in range(BL):
                nc.vector.memset(Hb[b][:, 0:1], 0.0)   # h0 = 0

            # ---- constants ----
            # khp/b128 (needed by the first projections) + the P12
            # dead-lane zeros ride gpsimd; the late-needed sweep weights
            # ride the scalar ring ahead of the remaps. The sync ring is
            # dedicated to the 16 tx loads (a remap interleaved with loads
            # head-of-line blocks the ring on its drain semaphore).
            khp_sb = [consts.tile([2 * D, 60], F16, tag=f"khp3_{q}",
                                  name=f"khp3_{q}")
                      for q in range(3)]
            b60_sb = consts.tile([60, 1], F32, tag="b60")
            bdrf_sb = consts.tile([P, P], MM_DT, tag="bdrf")
            bdrh_sb = consts.tile([P, P], MM_DT, tag="bdrh")
            id_sb = consts.tile([P, P], MM_DT, tag="ident")
            m2_sb = consts.tile([P + 1, G], F16, tag="m2")
            fcw_sb = consts.tile([P, 4], F16, tag="fcw125")
            for q in range(3):
                nc.gpsimd.dma_start(khp_sb[q][:], khp3[q])
            nc.gpsimd.dma_start(b60_sb[:], b60[:])
            # dead lanes (g >= NB[bl]) must be ZERO: the block-diag matmuls
            # multiply every lane by the weight column (0 * NaN = NaN would
            # pollute live psum rows). DMA-zeroed (engine memsets cost
            # ~5.4us of early DVE time; DMA partition bases are free).
            nc.gpsimd.dma_start(P12[0][5 * NB[0]:P, :], zer[0:P - 5 * NB[0]])
            nc.gpsimd.dma_start(P12[1][5 * NB[1]:P, :], zer[0:P - 5 * NB[1]])
            nc.gpsimd.dma_start(P12[2][5 * NB[2]:P, :], zer[0:P - 5 * NB[2]])
            # late-needed sweep weights ride the sync ring AFTER the loads
            # (sync is otherwise idle then; the scalar ring must stay empty
            # so block 0's remaps and the sweep ACT ops issue promptly)
            def emit_late_consts():
                nc.sync.dma_start(id_sb[:], ident[:])
                nc.sync.dma_start(bdrf_sb[:], bd_rf[:])
                nc.sync.dma_start(bdrh_sb[:], bd_rh[:])
                nc.sync.dma_start(m2_sb[:], m2[:])
                nc.sync.dma_start(fcw_sb[:], fcw125[:])
                for b in range(BL):
                    nc.sync.dma_start(rhs2[b][P:P + 1, :], fcb[:])
            # head rhs tiles: rows 0..124 written per block at the final
            # sweep; row 125 = fc_b (ones row of m2 adds the bias)
            rhs2 = [head_pool.tile([P + 1, 4], F16, tag=f"rhs2_{b}",
                                   name=f"rhs2_{b}") for b in range(BL)]
            # preload the Exp ACT table now (1.28us); otherwise it loads
            # lazily right before the head's exp, on the tail critical path
            exd = head_pool.tile([1, 1], F32, tag="exd")
            nc.scalar.activation(exd[:], b60_sb[0:1, 0:1], AF.Exp)

            # ---- Phase 1: plain transposed loads + projection ----
            # Loads are emitted per block (see the emission loop below):
            # the DMA engines are a single globally-serialized resource
            # (~650ns issue + bytes/360GBps per instruction), so block 0's
            # remaps must not queue behind later blocks' loads.
            xt2s = {}

            def emit_loads(bl):
                for qq in range(*([0, 6], [6, 12], [12, 16])[bl]):
                    xt = xt_pool.tile([2 * D, 2 * T], F16, tag="xt",
                                      name="xt")
                    nc.sync.dma_start(out=xt[:], in_=txpt[qq])
                    xt2s[qq] = xt

            def xt_slice(q, th):
                return xt2s[q // 2][:, (q % 2) * T + th * 512:
                                    (q % 2) * T + th * 512 + 512]

            remap_cnt = [0]

            def emit_group(grp):
                bl = GRP_BL[grp]
                g0 = GRP_G0[grp]
                np_ = GRP_NP[grp]
                q0 = 3 * grp
                nrow = 20 * np_
                stg = stg_pool.tile([60, 2 * 512], F16, tag="stg")
                for th in range(2):
                    ps = ps1_pool.tile([60, 512], F32, tag="psA")
                    # the 3 pairs ACCUMULATE into one compact [60, 512]
                    # region: khp3[ql] is zero outside cols 20ql..20ql+20
                    for ql in range(np_):
                        nc.tensor.matmul(
                            ps[:nrow, :],
                            lhsT=khp_sb[ql][:, :nrow],
                            rhs=xt_slice(q0 + ql, th),
                            start=(ql == 0), stop=(ql == np_ - 1),
                        )
                    # drains on DVE: keeps the scalar queue free for the
                    # sweep activations
                    nc.vector.tensor_scalar(
                        stg[:nrow, 512 * th:512 * th + 512], ps[:nrow, :],
                        b60_sb[:nrow, :], None, ALU.add)
                # ONE remap per group (11 total; each DMA instruction costs
                # ~650-784ns serial ring issue + ~0.7us transfer):
                # src row 20*ql + 2*(5*b01+u) + gate, free (th, t) ->
                # P12[bl] partition 5*(g0 + 2*ql + b01) + u,
                # free col gate*1024 + th*512 + t.
                s_ap = stg[:nrow, :]
                d_ap = (P12[bl][5 * g0:5 * g0 + 10 * np_, :]
                        .rearrange("p (gate tt t) -> p gate tt t",
                                   gate=2, tt=2))
                # block 0's remaps on the otherwise-empty scalar ring so
                # nothing delays them; later blocks' on gpsimd
                eng = nc.scalar if bl == 0 else nc.gpsimd
                remap_cnt[0] += 1
                eng.dma_start(out=d_ap, in_=s_ap)

            # ---- Phase 2 helpers ----
            def mm_pair(ps_t, w_sb, rhs_full, start, wl):
                # accumulate w_sb.T @ rhs into ps_t ([P, wl]) in <=512-col
                # chunks (a 1024-col matmul crosses a psum bank -> illegal)
                for c0 in range(0, wl, 512):
                    sl = slice(c0, min(c0 + 512, wl))
                    nc.tensor.matmul(ps_t[:, sl], lhsT=w_sb[:],
                                     rhs=rhs_full[:, sl],
                                     start=start, stop=not start)

            def scan_block(bl, w, m, w0):
                # h[t] = w[t]*h[t-1] + m[t] over t in [w0, T), fp32 state
                # (DVE only). 2x512 chunks (one 1024-col scan runs at 4
                # cyc/col vs 2.5); Hb col 0 is the memset h0=0.
                wl = T - w0
                for c0 in range(0, wl, 512):
                    c1 = min(c0 + 512, wl)
                    nc.vector.tensor_tensor_scan(
                        Hb[bl][:, w0 + c0 + 1:w0 + c1 + 1],
                        w[:, c0:c1], m[:, c0:c1],
                        Hb[bl][:, w0 + c0:w0 + c0 + 1],
                        ALU.mult, ALU.add)

            def emit_head(bl):
                # logits = M2.T @ (fcw125 * h_T ++ fc_b); per-block head +
                # softmax + out DMA so block 0's output leaves while block
                # 2 is still scanning (the shared tail measured ~7us).
                # (tensor_scalar wants an f32 scalar AP -> cast h_T col)
                hcol = head_pool.tile([P, 1], F32, tag=f"hcol_{bl}",
                                      name=f"hcol_{bl}")
                nc.vector.tensor_scalar(hcol[:], Hb[bl][:, T:T + 1],
                                        1.0, None, ALU.mult)
                nc.vector.tensor_scalar(rhs2[bl][0:P, :], fcw_sb[:],
                                        hcol[:], None, ALU.mult)
                pl = ps1_pool.tile([G, 4], F32, tag="psA", name=f"pl_{bl}")
                nc.tensor.matmul(pl[:], lhsT=m2_sb[:], rhs=rhs2[bl][:],
                                 start=True, stop=True)
                # |logits| < ~3: exp cannot overflow f32 -> skip max-shift
                ex = head_pool.tile([G, 4], F32, tag=f"ex_{bl}",
                                    name=f"ex_{bl}")
                sm = head_pool.tile([G, 1], F32, tag=f"sm_{bl}",
                                    name=f"sm_{bl}")
                nc.scalar.activation(ex[:], pl[:], AF.Exp, accum_out=sm[:])
                ri = head_pool.tile([G, 1], F32, tag=f"ri_{bl}",
                                    name=f"ri_{bl}")
                nc.vector.reciprocal(ri[:], sm[:])
                op = head_pool.tile([G, 4], F32, tag=f"op_{bl}",
                                    name=f"op_{bl}")
                nc.vector.tensor_scalar(op[:], ex[:], ri[:], None, ALU.mult)
                eng = (nc.sync, nc.scalar, nc.gpsimd)[bl]
                eng.dma_start(out=out[B0[bl]:B0[bl] + NB[bl], :],
                              in_=op[0:NB[bl], :])

            def emit_sweep0(bl):
                # sweep 0: h == 0 -> pa = P1, pb = P2, no matmuls
                v1 = gv1_pool.tile([P, T], F16, tag="v1", name="v1")
                nc.scalar.activation(v1[:], P12[bl][:, 0:T], AF.Sigmoid)
                v2 = gv2_pool.tile([P, T], F16, tag="v2", name="v2")
                nc.scalar.activation(v2[:], P12[bl][:, T:2 * T], AF.Tanh)
                w = gw_pool.tile([P, T], F16, tag="w", name="w")
                nc.scalar.activation(w[:], v1[:], AF.Copy,
                                     bias=1.0, scale=-1.0)
                m = gm_pool.tile([P, T], F16, tag="m", name="m")
                nc.vector.tensor_tensor(m[:], v1[:], v2[:], ALU.mult)
                scan_block(bl, w, m, 0)

            # ---- emission ----
            # Block-major phase 1 + sweep 0: block bl's sweep-0 unit starts
            # as soon as its last remap lands, overlapping later blocks'
            # loads/projections.
            for bl in range(BL):
                emit_loads(bl)
                if bl == 0:
                    # sweep weights slot in behind block 0's loads: tiny
                    # transfers, and the sync issue delay for later loads
                    # hides behind the serialized DMA device anyway
                    emit_late_consts()
                for grp in BL_GRPS[bl]:
                    emit_group(grp)
                emit_sweep0(bl)

            # Lockstep sweeps with cross-block weight batching (PE
            # pipelining; per-unit emission measured 60us slower on v2).
            for s in range(1, NSWEEPS):
                w0 = W0S[s]
                wl = T - w0
                # per-block MM interleave: pa[bl] completes after ITS 4
                # MMs instead of waiting the whole cross-block batch (the
                # batched order made pb[0] transitively wait on hv[2],
                # stretching the sweep cadence to ~15.5us vs ~11us busy)
                pa = [ps2_pool.tile([P, wl], F32, tag="ps2", name="pa")
                      for _ in range(BL)]
                for bl in range(BL):
                    mm_pair(pa[bl], bdrf_sb, Hb[bl][:, w0:T], True, wl)
                    mm_pair(pa[bl], id_sb, P12[bl][:, w0:T], False, wl)
                v1s, ws, hvs = [], [], []
                for bl in range(BL):
                    v1 = gv1_pool.tile([P, wl], F16, tag="v1", name="v1")
                    nc.scalar.activation(v1[:], pa[bl][:], AF.Sigmoid)
                    v1s.append(v1)
                    hv = ghv_pool.tile([P, wl], F16, tag="hv", name="hv")
                    nc.vector.tensor_tensor(hv[:], Hb[bl][:, w0:T], v1[:],
                                            ALU.mult)
                    hvs.append(hv)
                pb = [ps2_pool.tile([P, wl], F32, tag="ps2", name="pb")
                      for _ in range(BL)]
                for bl in range(BL):
                    mm_pair(pb[bl], bdrh_sb, hvs[bl][:], True, wl)
                    mm_pair(pb[bl], id_sb, P12[bl][:, T + w0:2 * T],
                            False, wl)
                for bl in range(BL):
                    v2 = gv2_pool.tile([P, wl], F16, tag="v2", name="v2")
                    nc.scalar.activation(v2[:], pb[bl][:], AF.Tanh)
                    # w = 1 - v1 as an ACT Copy (cannot read pa here: its
                    # psum buffer is already recycled into pb). On ACT
                    # right after the same block's tanh: off the critical
                    # chain (only the scan reads it); gpsimd measured
                    # worse (shared SBUF port contention slowed DVE ~35%)
                    w = gw_pool.tile([P, wl], F16, tag="w", name="w")
                    nc.scalar.activation(w[:], v1s[bl][:], AF.Copy,
                                         bias=1.0, scale=-1.0)
                    ws.append(w)
                    m = gm_pool.tile([P, wl], F16, tag="m", name="m")
                    nc.vector.tensor_tensor(m[:], v1s[bl][:], v2[:],
                                            ALU.mult)
                    scan_block(bl, ws[bl], m, w0)
                    if s == NSWEEPS - 1:
                        emit_head(bl)

            if dbg:
                for b in range(BL):
                    nc.gpsimd.dma_start(out=p12d[b][:], in_=P12[b][:])
                    nc.gpsimd.dma_start(out=hbd[b][:], in_=Hb[b][:])

    nc.compile()
    return nc


def _prep_host_inputs(kernel, rec_kernel, bias, fc_w, fc_b):
    f32 = np.float32
    k = np.asarray(kernel, f32).astype(np.float16)    # [64, 10]

    # compact psum row = 20*ql + 2*(5*b01 + u) + gate (gate innermost so
    # the remap DMA sees contiguous gate pairs); khp3[ql] zero-padded so
    # the three pair-matmuls accumulate into one [60, 512] psum region
    khp3 = np.zeros((3, 2 * D, 60), np.float16)
    b60 = np.zeros((60, 1), f32)
    bias_f = np.asarray(bias, f32)
    for gate in range(2):
        for b01 in range(2):
            for u in range(U):
                c = 2 * (5 * b01 + u) + gate
                for ql in range(3):
                    khp3[ql, D * b01:D * b01 + D, 20 * ql + c] = \
                        k[:, 5 * gate + u]
                    b60[20 * ql + c, 0] = bias_f[5 * gate + u]

    rk = np.asarray(rec_kernel, f32)
    bd_rf = np.zeros((P, P), np.float16)
    bd_rh = np.zeros((P, P), np.float16)
    for g in range(G):
        bd_rf[5 * g:5 * g + 5, 5 * g:5 * g + 5] = rk[:, :U]
        bd_rh[5 * g:5 * g + 5, 5 * g:5 * g + 5] = rk[:, U:]
    ident = np.eye(P, dtype=np.float16)

    # head selector: logits[g, j] = sum_u h[5g+u] fc_w[u, j] + fc_b[j]
    m2 = np.zeros((P + 1, G), np.float16)
    for g in range(G):
        m2[5 * g:5 * g + 5, g] = 1.0
    m2[P, :] = 1.0
    fcw125 = np.tile(np.asarray(fc_w, f32), (G, 1)).astype(np.float16)
    fcb = np.asarray(fc_b, f32).reshape(1, 4).astype(np.float16)
    zer = np.zeros((45, 2 * T), np.float16)
    return dict(khp3=khp3, b60=b60, bd_rf=bd_rf, bd_rh=bd_rh, ident=ident,
                m2=m2, fcw125=fcw125, fcb=fcb, zer=zer)


_CACHE = {}


def kernel(tx, kernel, rec_kernel, bias, fc_w, fc_b, _want_time=False):
    tx = np.asarray(tx, np.float32)
    host = _prep_host_inputs(kernel, rec_kernel, bias, fc_w, fc_b)

    # fp16 pre-transposed tx: [core, load, (b01, d), (q_lo, t)]
    # load qq covers pairs 2qq, 2qq+1; pair pq covers batches 2pq, 2pq+1.
    txpt_all = np.ascontiguousarray(
        tx.reshape(NCORES, NLOAD, 2, 2, T, D)    # c, qq, q_lo, b01, t, d
        .transpose(0, 1, 3, 5, 2, 4)             # c, qq, b01, d, q_lo, t
        .reshape(NCORES, NLOAD, 2 * D, 2 * T).astype(np.float16))

    if "nc" not in _CACHE:
        _CACHE["nc"] = build_program()
    nc = _CACHE["nc"]

    in_maps = []
    for c in range(NCORES):
        m = {"txpt": txpt_all[c]}
        m.update(host)
        in_maps.append(m)

    try:
        res = run_bass_kernel_spmd(
            nc, in_maps, core_ids=list(range(NCORES)), trace=_want_time
        )
    except ModuleNotFoundError:
        res = run_bass_kernel_spmd(
            nc, in_maps, core_ids=list(range(NCORES)), trace=False
        )
    outs = [res.results[c]["out"] for c in range(NCORES)]
    full = np.concatenate(outs, axis=0)
    if _want_time:
        _CACHE["res"] = res
        return full, res.exec_time_ns
    return full
